# revision 48
# baseline (speedup 1.0000x reference)
"""Causal self-attention (B=4, S=2048, D=1024, H=16) on 8 TRN2 NeuronCores.

Sharding: core = (batch b, head-group g) with b = core//2, g = core%2.
Each core computes, for its batch and its 8 heads: QKV projection,
transposed flash-style attention (scores kept k-major so the softmax'd
weights feed the PV matmul directly as the moving operand), and a partial
output projection over its heads' 512 hidden dims.  The host sums the two
partial projections per batch.

Perf notes (v4):
- x is transposed on the PE in d-major units (all four seq-tiles of one
  dh-pair per unit), so the first Q-projection granule's moving operand is
  complete after one unit (~0.5us of PE).  DMA-crossbar transposes were
  tried and are ~10x slower than the cost model claims on this hardware.
- Startup streams x on both hwdge queues and weights on the scalar queue
  in parallel; transpose/projection units rotate across the idle attention
  psum pools so nothing serializes on a single psum bank.
- Background PE work is emitted in ~2-matmul granules BETWEEN a group's
  score matmuls and its PV matmuls (the PV waits on exp; granules emitted
  after it would head-of-line block the in-order PE queue).  TRN2 drops the
  PE clock from 2.4 to 1.2 GHz for ~3us after every idle gap, so the PE
  must never go idle.
- exp() extents are clipped on the causal-diagonal chunk pairs.
- The softmax denominator never touches a psum scatter: each head's ones-row
  is copied (DVE for even rows, small SBUF-to-SBUF DMA for odd rows - DMA
  writes have no 32-partition alignment rule) into a per-head-pair [2,512]
  SBUF tile, and each pair is normalized as soon as both heads finish
  (heads run odd-first within a pair so no DMA sits on the reciprocal path).
- All output projections are deferred into window 3, which is otherwise
  exp(ACT)-bound.
- Output is stored fp16 (host accumulates partials in fp32).

Numerics: matmul operands are fp16 (1 cycle/row on the PE vs 2 for fp32r,
accumulation still fp32 in PSUM); exp runs with a constant -4 shift, tuned so
fp16 softmax weights neither overflow nor hit the subnormal flush-to-zero
floor (the shift cancels in normalization).
End-to-end relative error ~1e-3 vs the fp32 reference.
"""

import numpy as np

B, S, D, H = 4, 2048, 1024, 16
HS = 64            # head size
NH = 8             # heads per core
C = 512            # per-core q/k/v width (NH * HS)
P = 128
NCORES = 8
DCH = D // P       # 8 contraction chunks for the projections
NW = S // 512      # 4 query windows of 512
KCH = S // P       # 16 key chunks
ESHIFT = -4.0      # exp(0.25*s + ESHIFT); cancels in the softmax ratio

_cache = {}


def _build(has_bias=False):
    key = ("nc", has_bias)
    if key in _cache:
        return _cache[key]

    from contextlib import ExitStack

    import concourse.bass as bass
    import concourse.tile as tile
    from concourse import bacc, mybir
    from concourse.masks import make_identity, make_upper_triangular

    f32 = mybir.dt.float32
    f16 = mybir.dt.float16
    Exp = mybir.ActivationFunctionType.Exp

    nc = bacc.Bacc(
        "TRN2", target_bir_lowering=False, debug=False, num_devices=NCORES
    )

    x_d = nc.dram_tensor("x", [S, D], f16, kind="ExternalInput").ap()
    wq_d = nc.dram_tensor("wq", [D, C], f16, kind="ExternalInput").ap()
    wk_d = nc.dram_tensor("wk", [D, C], f16, kind="ExternalInput").ap()
    wv_d = nc.dram_tensor("wv", [D, C], f16, kind="ExternalInput").ap()
    wp_d = nc.dram_tensor("wp", [C, D], f16, kind="ExternalInput").ap()
    # per-partition bias columns: col j<4 -> q col-tile j, col 4+j -> k col-tile j
    bqk_d = nc.dram_tensor("bqk", [P, 8], f32, kind="ExternalInput").ap()
    bv_d = nc.dram_tensor("bv", [1, C], f32, kind="ExternalInput").ap()
    bp_d = nc.dram_tensor("bp", [1, D], f32, kind="ExternalInput").ap()
    out_d = nc.dram_tensor("out", [S, D], f16, kind="ExternalOutput").ap()

    with tile.TileContext(nc) as tc, ExitStack() as ctx:
        ctx.enter_context(nc.allow_low_precision(reason="fp16 attention"))

        const = ctx.enter_context(tc.tile_pool(name="const", bufs=1))
        persist = ctx.enter_context(tc.tile_pool(name="persist", bufs=1))
        xload = ctx.enter_context(tc.tile_pool(name="xload", bufs=8))
        xtp = ctx.enter_context(tc.tile_pool(name="xtp", bufs=2))
        qtw = ctx.enter_context(tc.tile_pool(name="qtw", bufs=2))
        otw = ctx.enter_context(tc.tile_pool(name="otw", bufs=4))
        expool = ctx.enter_context(tc.tile_pool(name="expool", bufs=4))
        denpool = ctx.enter_context(tc.tile_pool(name="denpool", bufs=3))
        dpairs = ctx.enter_context(tc.tile_pool(name="dpairs", bufs=8))
        rhpool = ctx.enter_context(tc.tile_pool(name="rhpool", bufs=4))
        stpool = ctx.enter_context(tc.tile_pool(name="stpool", bufs=3))
        prstage = ctx.enter_context(tc.tile_pool(name="prstage", bufs=8))

        # pS holds one score group per head of the active pair (slots E/O);
        # pPV holds both heads' PV accumulators
        pS = ctx.enter_context(tc.tile_pool(name="pS", bufs=2, space="PSUM"))
        pPV = ctx.enter_context(tc.tile_pool(name="pPV", bufs=2, space="PSUM"))
        pMISC = ctx.enter_context(tc.tile_pool(name="pMISC", bufs=1, space="PSUM"))
        pAUX = ctx.enter_context(tc.tile_pool(name="pAUX", bufs=1, space="PSUM"))

        identf = const.tile([P, P], f32, tag="identf")
        make_identity(nc, identf)
        ident = const.tile([P, P], f16, tag="ident")
        nc.vector.tensor_copy(ident, identf)
        tri = const.tile([P, P], f32, tag="tri")
        make_upper_triangular(nc, tri, val=1.0, diag=True)  # tri[k,q]=1 iff q>=k
        eshift_sb = const.tile([P, 1], f32, tag="eshift")
        nc.vector.memset(eshift_sb, ESHIFT)
        ones8 = const.tile([P, 8], f32, tag="ones8")
        nc.vector.memset(ones8, 1.0)
        # head-pair band select: within any 128-col tile j, the first 64 cols
        # belong to head 2j (rh row 0), the last 64 to head 2j+1 (row 1)
        bandf = const.tile([2, 128], f32, tag="bandf")
        nc.gpsimd.memset(bandf, 1.0)
        nc.gpsimd.affine_select(
            out=bandf, in_=bandf, compare_op=mybir.AluOpType.is_ge,
            fill=0.0, base=0, pattern=[[1, 128]], channel_multiplier=-64)
        nc.gpsimd.affine_select(
            out=bandf, in_=bandf, compare_op=mybir.AluOpType.is_ge,
            fill=0.0, base=63, pattern=[[-1, 128]], channel_multiplier=64)
        sel_p = const.tile([2, 128], f16, tag="sel_p")
        nc.vector.tensor_copy(sel_p, bandf[:])

        if has_bias:
            bqk_sb = const.tile([P, 8], f32, tag="bqk")
            nc.sync.dma_start(bqk_sb, bqk_d)
            bv_bc = const.tile([P, C], f32, tag="bv_bc")
            nc.sync.dma_start(
                bv_bc,
                bass.AP(tensor=bv_d.tensor, offset=bv_d.offset,
                        ap=[[0, P], list(bv_d.ap[-1])]),
            )
            bp_bc = const.tile([P, D], f32, tag="bp_bc")
            nc.sync.dma_start(
                bp_bc,
                bass.AP(tensor=bp_d.tensor, offset=bp_d.offset,
                        ap=[[0, P], list(bp_d.ap[-1])]),
            )

        wq_sb = persist.tile([P, DCH, C], f16, tag="wq")
        wk_sb = persist.tile([P, DCH, C], f16, tag="wk")
        wv_sb = persist.tile([P, DCH, C], f16, tag="wv")
        wp_sb = persist.tile([P, 4, D], f16, tag="wp")
        KT = persist.tile([P, 4, S], f16, tag="KT")
        Vt = persist.tile([P, KCH, NH * 65], f16, tag="Vt")

        # ---------- unit generators (emitted lazily for interleaving) ----------

        def x_dma(w, xts, eng=None):
            """Plain DMA of window w's x rows into 4 seq-tiles [P, D]."""
            eng = eng or nc.sync
            for st in range(4):
                row0 = (4 * w + st) * P
                eng.dma_start(xts[st], x_d[row0:row0 + P, :])

        def xT_granules(w, xts, xT, pools=None):
            """PE-transpose window w's x into xT [P, DCH, 512], one dh-PAIR
            per granule (all 4 seq-tiles), so consumers needing low dh chunks
            unblock after a single granule.  psum slot order (dhh, st)."""
            pools = pools or [pAUX]
            for jd in range(4):
                pool = pools[jd % len(pools)]
                def unit(w=w, jd=jd, xts=xts, xT=xT, pool=pool):
                    ptr = pool.tile([P, 2, 4, P], f16, name="tr_ps",
                                    tag="aux" if pool is pAUX
                                    else ("pv" if pool is pPV else "misc"))
                    for dhh in range(2):
                        d = 2 * jd + dhh
                        for st in range(4):
                            nc.tensor.transpose(
                                ptr[:, dhh, st, :],
                                xts[st][:, d * P:(d + 1) * P], ident[:])
                    nc.vector.tensor_copy(
                        xT[:, 2 * jd:2 * jd + 2, :]
                        .rearrange("p a (b c) -> p a b c", c=P), ptr[:])
                yield unit

        def qk_granules(w, xT, qt, pools=None, order=None):
            """Q^T / K^T projections for window w from xT, in 2-matmul granules.
            Order: ct-major with q before k so head 2ct/2ct+1's inputs complete
            as early as possible."""
            pools = pools or [pAUX]
            units = order or [(ct, qk) for ct in range(4) for qk in range(2)]
            for ui, (ct, qk) in enumerate(units):
                ps = [None]
                pool = pools[ui % len(pools)]
                for gd in range(4):
                    def unit(w=w, ct=ct, qk=qk, gd=gd, xT=xT, qt=qt, ps=ps,
                             pool=pool):
                        if gd == 0:
                            ps[0] = pool.tile([P, 512], f32, name="aux_ps",
                                              tag="aux" if pool is pAUX
                                              else "misc")
                        wsb = wq_sb if qk == 0 else wk_sb
                        for d in (2 * gd, 2 * gd + 1):
                            nc.tensor.matmul(
                                ps[0], wsb[:, d, ct * P:(ct + 1) * P],
                                xT[:, d, :],
                                start=(d == 0), stop=(d == DCH - 1))
                        if gd == 3:
                            dest = (qt[:, ct, :] if qk == 0
                                    else KT[:, ct, w * 512:(w + 1) * 512])
                            if has_bias:
                                nc.vector.tensor_scalar_add(
                                    dest, ps[0],
                                    bqk_sb[:, qk * 4 + ct:qk * 4 + ct + 1])
                            else:
                                nc.vector.tensor_copy(dest, ps[0])
                    yield unit

        def v_granules(w, xT, pools=None):
            """V for the 4 key chunks of window w, head-grouped with ones col."""
            pools = pools or [pAUX]
            for st in range(4):
                ps = [None]
                pool = pools[st % len(pools)]
                for gd in range(4):
                    def unit(w=w, st=st, gd=gd, xT=xT, ps=ps, pool=pool):
                        if gd == 0:
                            ps[0] = pool.tile([P, 512], f32, name="aux_ps",
                                              tag="aux" if pool is pAUX
                                              else "misc")
                        for d in (2 * gd, 2 * gd + 1):
                            nc.tensor.matmul(ps[0],
                                             xT[:, d, st * P:(st + 1) * P],
                                             wv_sb[:, d, :],
                                             start=(d == 0), stop=(d == DCH - 1))
                        if gd == 3:
                            kc = 4 * w + st
                            vtv = Vt[:, kc, :].rearrange("p (h c) -> p h c", c=65)
                            if has_bias:
                                nc.vector.tensor_add(
                                    vtv[:, :, 0:64],
                                    ps[0].rearrange("p (h c) -> p h c", c=64),
                                    bv_bc[:].rearrange("p (h c) -> p h c", c=64))
                            else:
                                nc.vector.tensor_copy(
                                    vtv[:, :, 0:64],
                                    ps[0].rearrange("p (h c) -> p h c", c=64))
                            nc.vector.tensor_copy(vtv[:, :, 64:65],
                                                  ones8[:, :].unsqueeze(2))
                    yield unit

        def proj_granules(w, ot, pools=None):
            """Output projection for window w's 512 seq rows (partial over C),
            2-matmul granules; the closing granule stages and DMAs out fp16."""
            pools = pools or [pMISC]
            for ui in range(8):
                st, cw = ui // 2, ui % 2
                pp = [None]
                pool = pools[ui % len(pools)]
                for gj in range(2):
                    def unit(w=w, st=st, cw=cw, gj=gj, ot=ot, pp=pp, pool=pool):
                        if gj == 0:
                            pp[0] = pool.tile([P, 512], f32, name="proj_pp",
                                              tag="aux" if pool is pAUX
                                              else ("pv" if pool is pPV
                                                    else "misc"))
                        for j in (2 * gj, 2 * gj + 1):
                            nc.tensor.matmul(
                                pp[0], ot[:, j, st * P:(st + 1) * P],
                                wp_sb[:, j, cw * 512:(cw + 1) * 512],
                                start=(j == 0), stop=(j == 3))
                        if gj == 1:
                            stg = stpool.tile([P, 512], f16, tag="stg")
                            if has_bias:
                                nc.vector.tensor_add(
                                    stg, pp[0],
                                    bp_bc[:, cw * 512:(cw + 1) * 512])
                            else:
                                nc.vector.tensor_copy(stg, pp[0])
                            row0 = (4 * w + st) * P
                            nc.sync.dma_start(
                                out_d[row0:row0 + P,
                                      cw * 512:(cw + 1) * 512], stg)
                    yield unit

        def proj_split_units(w, ot, stage_tiles, phase, pools=None):
            """Split projection for the final window: phase 0 accumulates
            contraction chunks j=0,1 (heads 0-3, normalized by mid-window)
            and stages fp16 partials; phase 1 adds chunks j=2,3 and fuses the
            partial back in with the staging copy, shortening the tail."""
            pools = pools or [pMISC]
            for ui in range(8):
                st, cw = ui // 2, ui % 2
                pool = pools[ui % len(pools)]
                def unit(w=w, st=st, cw=cw, phase=phase, ot=ot, pool=pool,
                         ui=ui):
                    pp = pool.tile([P, 512], f32, name="proj_pp",
                                   tag="aux" if pool is pAUX
                                   else ("pv" if pool is pPV else "misc"))
                    for j in (2 * phase, 2 * phase + 1):
                        nc.tensor.matmul(
                            pp, ot[:, j, st * P:(st + 1) * P],
                            wp_sb[:, j, cw * 512:(cw + 1) * 512],
                            start=(j == 2 * phase), stop=(j == 2 * phase + 1))
                    if phase == 0:
                        nc.vector.tensor_copy(stage_tiles[ui], pp)
                    else:
                        stg = stpool.tile([P, 512], f16, tag="stg")
                        nc.vector.tensor_add(stg, pp, stage_tiles[ui])
                        if has_bias:
                            nc.vector.tensor_add(
                                stg, stg, bp_bc[:, cw * 512:(cw + 1) * 512])
                        row0 = (4 * w + st) * P
                        nc.sync.dma_start(
                            out_d[row0:row0 + P, cw * 512:(cw + 1) * 512], stg)
                yield unit

        # debt-based background filler -------------------------------------
        PE_NS = 0.55             # measured ns per matmul row (sustained)
        GRAN_NS = 600.0          # ~2 fp16 matmuls of 512 rows, measured
        ACT_OH = 262.0           # measured ACT fixed overhead per instruction
        state = {"debt": 0.0, "bg": []}

        def fill(act_ns, pe_ns):
            """Inject background granules to cover PE idle while the scalar
            engine runs exp (or a cross-engine dependency settles)."""
            state["debt"] += act_ns - pe_ns
            while state["debt"] > 0.5 * GRAN_NS and state["bg"]:
                state["bg"].pop(0)()
                state["debt"] -= GRAN_NS
            if state["debt"] < -3 * GRAN_NS:
                state["debt"] = -3 * GRAN_NS

        first_pair = [True]

        def attention_pair(w, j, qt, ot, den_pair):
            """Scores+exp+PV for head pair (2j, 2j+1).  The two heads' score
            matmuls use the same KT/qt columns at partition offsets 0 and 64,
            so interleaving them dual-issues on the PE's two 64-row groups
            (measured 2.7x on a score-matmul stream).  Unnormalized O -> ot,
            denominator rows -> den_pair (DVE row 0, small DMA row 1)."""
            hE, hO = 2 * j, 2 * j + 1
            ct = j
            pvE = pPV.tile([65, 512], f32, tag="pv", name="pvE")
            pvO = pPV.tile([65, 512], f32, tag="pv", name="pvO")
            last_kc = 4 * w + 3
            first = first_pair[0]
            first_pair[0] = False
            for g2 in range(2 * (w + 1)):      # 2-chunk half groups
                scE = pS.tile([P, 2, 512], f32, tag="sc", name="scE")
                scO = pS.tile([P, 2, 512], f32, tag="sc", name="scO")
                pe_ns = 0.0
                for rr in range(2):
                    kc = 2 * g2 + rr
                    # masked q-range is never read downstream; keep the very
                    # first pair full-extent so all psum slots initialize
                    s0 = 0 if first else max(0, kc - 4 * w) * P
                    for sc, po in ((scE, 0), (scO, 64)):
                        nc.tensor.matmul(
                            sc[:, rr, s0:],
                            KT[po:po + 64, ct, kc * P:(kc + 1) * P],
                            qt[po:po + 64, ct, s0:],
                            start=True, stop=True)
                        pe_ns += (512 - s0) * 0.21   # dual-issued rate
                # clip exp on the upper diagonal pair: chunks (4w+2, 4w+3)
                # only feed queries >= 256 (their tri-masked PV extents)
                e0 = 0
                if not first and 2 * g2 == 4 * w + 2:
                    e0 = 256
                exE = expool.tile([P, 2, 512], f16, tag="ex", name="exE")
                exO = expool.tile([P, 2, 512], f16, tag="ex", name="exO")
                nc.scalar.activation(exE[:, :, e0:], scE[:, :, e0:], Exp,
                                     scale=0.25, bias=eshift_sb[:])
                nc.scalar.activation(exO[:, :, e0:], scO[:, :, e0:], Exp,
                                     scale=0.25, bias=eshift_sb[:])
                act_e = (1024 - 2 * e0) * 0.853 + ACT_OH
                pad = 150.0
                if 2 * g2 + 1 >= 4 * w:
                    # diagonal chunks insert a DVE tri-mask between exp and
                    # PV; cover that extra cross-engine latency too
                    pad += 450.0
                # background fill BEFORE the PV matmuls: PV waits on exp, and
                # anything emitted after it would head-of-line block the PE.
                # First fill covers exp-E's latency against the (dual-issued,
                # cheap) scores; tri-masks and the E-head PV block follow.
                fill(act_e + pad, pe_ns)
                q0s = []
                for rr in range(2):
                    kc = 2 * g2 + rr
                    r = kc - 4 * w            # >=0 only inside the diag block
                    q0 = 0
                    if r >= 0:
                        # only q >= r*P can attend this chunk; clip the PV
                        # extent instead of zero-filling the masked region
                        q0 = r * P
                        nc.vector.tensor_mul(exE[:, rr, q0:q0 + P],
                                             exE[:, rr, q0:q0 + P], tri[:])
                        nc.vector.tensor_mul(exO[:, rr, q0:q0 + P],
                                             exO[:, rr, q0:q0 + P], tri[:])
                    q0s.append(q0)
                pv_rows = 0.0
                for rr in range(2):
                    kc = 2 * g2 + rr
                    nc.tensor.matmul(pvE[:, q0s[rr]:],
                                     Vt[:, kc, hE * 65:(hE + 1) * 65],
                                     exE[:, rr, q0s[rr]:],
                                     start=(kc == 0), stop=(kc == last_kc))
                    pv_rows += (512 - q0s[rr]) * 0.42
                # second fill covers exp-O's stagger behind exp-E before the
                # O-head PV block
                fill(act_e, pv_rows)
                for rr in range(2):
                    kc = 2 * g2 + rr
                    nc.tensor.matmul(pvO[:, q0s[rr]:],
                                     Vt[:, kc, hO * 65:(hO + 1) * 65],
                                     exO[:, rr, q0s[rr]:],
                                     start=(kc == 0), stop=(kc == last_kc))
            # stash unnormalized O scaled by 1/4 (fp16 range headroom)
            nc.vector.tensor_scalar_mul(ot[0:64, ct, :], pvE[0:64, :], 0.25)
            nc.vector.tensor_scalar_mul(ot[64:128, ct, :], pvO[0:64, :], 0.25)
            # even head -> pair row 0: direct DVE copy (aligned); odd head ->
            # row 1: partition 1 is unaligned for the DVE, so stage on
            # partition 0 and let a 2KB DMA place it
            nc.vector.tensor_copy(den_pair[0:1, :], pvE[64:65, :])
            den_h = denpool.tile([1, 512], f32, tag="den")
            nc.vector.tensor_copy(den_h, pvO[64:65, :])
            nc.sync.dma_start(den_pair[1:2, :], den_h)

        def pair_norm_units(j, ot, den_pair, bc_pool=None):
            """Normalize ot column-tile j (heads 2j, 2j+1) from den_pair."""
            bc_pool = bc_pool or pMISC
            shared = {}

            def recip_unit(den_pair=den_pair):
                rall = rhpool.tile([2, 512], f32, tag="rall")
                nc.vector.reciprocal_approx_fast(rall, den_pair[:])
                rh = rhpool.tile([2, 512], f16, tag="rh")
                # x4 so small reciprocals clear the fp16 subnormal floor; the
                # 4*(1/4) pair cancels in the final normalize multiply
                nc.vector.tensor_scalar_mul(rh, rall, 4.0)
                shared["rh"] = rh
            yield recip_unit

            def bcmul_unit(j=j, ot=ot, pool=bc_pool):
                rh = shared["rh"]
                bcp = pool.tile([P, 512], f32, name="bc_ps",
                                tag="aux" if pool is pAUX
                                else ("pv" if pool is pPV else "misc"))
                nc.tensor.matmul(bcp, sel_p[:, :], rh[:], start=True, stop=True)
                # multiply reads the broadcast factors straight from PSUM
                nc.vector.tensor_mul(ot[:, j, :], ot[:, j, :], bcp[:])
            yield bcmul_unit

        # ---------------------------- schedule ----------------------------

        qt_cur = qtw.tile([P, 4, 512], f16, tag="qt")
        xT_cur = xtp.tile([P, DCH, 512], f16, tag="xT", name="xT")
        # startup: x tiles split across both hwdge queues, then wq on the
        # scalar queue and wk on sync in parallel; transpose units rotate
        # through the idle attention psum pools and riffle with the first
        # projection granules so the PE is dense from ~2us on
        xts0 = [xload.tile([P, D], f16, tag="xt", name="xt") for _ in range(4)]
        nc.scalar.dma_start(xts0[0], x_d[0:P, :])
        nc.sync.dma_start(xts0[1], x_d[P:2 * P, :])
        nc.scalar.dma_start(xts0[2], x_d[2 * P:3 * P, :])
        nc.sync.dma_start(xts0[3], x_d[3 * P:4 * P, :])
        for d in range(DCH):
            nc.scalar.dma_start(wq_sb[:, d, :], wq_d[d * P:(d + 1) * P, :])
            nc.sync.dma_start(wk_sb[:, d, :], wk_d[d * P:(d + 1) * P, :])
        for d in range(DCH):
            nc.scalar.dma_start(wv_sb[:, d, :], wv_d[d * P:(d + 1) * P, :])
        for j in range(4):            # proj weights are not needed until w3
            nc.sync.dma_start(wp_sb[:, j, :], wp_d[j * P:(j + 1) * P, :])

        tr0 = list(xT_granules(0, xts0, xT_cur, pools=[pPV, pMISC]))
        qk0 = list(qk_granules(0, xT_cur, qt_cur, pools=[pAUX]))
        v0 = list(v_granules(0, xT_cur, pools=[pMISC]))
        tr0[0]()
        tr0[1]()
        qk0[0]()
        tr0[2]()
        qk0[1]()
        tr0[3]()
        for u in qk0[2:]:
            u()
        for u in v0:
            u()

        deferred_proj = []      # proj granules, all pushed into window 3
        last_norm = []
        for w in range(NW):
            ot_cur = otw.tile([P, 4, 512], f16, tag="ot")

            bg = []
            if w == NW - 1:
                bg += deferred_proj
                deferred_proj = []
            if w + 1 < NW:
                qt_next = qtw.tile([P, 4, 512], f16, tag="qt")
                xT_next = xtp.tile([P, DCH, 512], f16, tag="xT", name="xT")
                xts_n = [xload.tile([P, D], f16, tag="xt", name="xt")
                         for _ in range(4)]
                x_dma(w + 1, xts_n, eng=nc.sync)
                bg += list(xT_granules(w + 1, xts_n, xT_next, pools=[pAUX]))
                bg += list(qk_granules(w + 1, xT_next, qt_next))
                bg += list(v_granules(w + 1, xT_next, pools=[pAUX]))
            else:
                qt_next = xT_next = None
            state["bg"] = bg
            state["debt"] = 0.0

            den_tiles = [dpairs.tile([2, 512], f32, tag="dpair", name="dpair")
                         for _ in range(4)]
            if w == NW - 1:
                stage_tiles = [prstage.tile([P, 512], f16, tag="pstg",
                                            name="pstg") for _ in range(8)]
                partial3 = list(proj_split_units(w, ot_cur, stage_tiles, 0))
            for j in range(4):
                attention_pair(w, j, qt_cur, ot_cur, den_tiles[j])
                units = list(pair_norm_units(j, ot_cur, den_tiles[j]))
                if w == NW - 1 and j <= 1:
                    # final window: normalize heads 0-3 inline so the
                    # split projection's first phase (contraction j=0,1)
                    # can run as background work during pairs 2,3
                    for u in units:
                        u()
                    if j == 1:
                        state["bg"] = partial3 + state["bg"]
                elif (w, j) == (NW - 1, 3):
                    last_norm = units       # tail: run right after flush
                else:
                    state["bg"] = units + state["bg"]
            for u in state["bg"]:
                u()
            state["bg"] = []

            if w + 1 < NW:
                # deferred projections run inside window 3, where pAUX is
                # otherwise idle (norm/bc stay on pMISC, so pair-norm units
                # can be inserted mid-stream without psum aliasing)
                deferred_proj += list(proj_granules(w, ot_cur, pools=[pAUX]))
            else:
                for u in last_norm:
                    u()
                for u in proj_split_units(w, ot_cur, stage_tiles, 1,
                                          pools=[pMISC, pAUX]):
                    u()

            qt_cur, xT_cur = qt_next, xT_next

    nc.compile()
    _cache[key] = nc
    return nc


def _make_in_maps(input_data, w_qkv, b_qkv, w_proj, b_proj):
    x = np.asarray(input_data, dtype=np.float32).astype(np.float16)
    wqkv = np.asarray(w_qkv, dtype=np.float32).astype(np.float16)
    bqkv = np.asarray(b_qkv, dtype=np.float32)
    wp = np.asarray(w_proj, dtype=np.float32).astype(np.float16)
    bp = np.asarray(b_proj, dtype=np.float32)

    in_maps = []
    for core in range(NCORES):
        b, g = core // 2, core % 2
        cs = slice(g * C, (g + 1) * C)
        bq = bqkv[0 * D:1 * D][cs]
        bk = bqkv[1 * D:2 * D][cs]
        bqk = np.empty((P, 8), np.float32)
        for j in range(4):
            bqk[:, j] = bq[j * P:(j + 1) * P]
            bqk[:, 4 + j] = bk[j * P:(j + 1) * P]
        in_maps.append({
            "x": np.ascontiguousarray(x[b]),
            "wq": np.ascontiguousarray(wqkv[:, 0 * D:1 * D][:, cs]),
            "wk": np.ascontiguousarray(wqkv[:, 1 * D:2 * D][:, cs]),
            "wv": np.ascontiguousarray(wqkv[:, 2 * D:3 * D][:, cs]),
            "wp": np.ascontiguousarray(wp[cs, :]),
            "bqk": bqk,
            "bv": np.ascontiguousarray(bqkv[2 * D:3 * D][cs]).reshape(1, C),
            "bp": (bp if g == 0 else np.zeros_like(bp)).reshape(1, D),
        })
    return in_maps


def kernel(input_data, w_qkv, b_qkv, w_proj, b_proj):
    from concourse.bass_utils import run_bass_kernel_spmd

    has_bias = bool(np.any(np.asarray(b_qkv)) or np.any(np.asarray(b_proj)))
    nc = _build(has_bias)
    in_maps = _make_in_maps(input_data, w_qkv, b_qkv, w_proj, b_proj)
    res = run_bass_kernel_spmd(nc, in_maps, core_ids=list(range(NCORES)))
    parts = [np.asarray(res.results[i]["out"], dtype=np.float32)
             for i in range(NCORES)]
    out = np.stack([parts[2 * b] + parts[2 * b + 1] for b in range(B)])
    return out.astype(np.float32)
